# revision 16
# baseline (speedup 1.0000x reference)
"""Trainium2 Bass kernel for nn_Attention_85882166051391.

Reference computation (per batch b):
    k = (memory @ Wk)  viewed as 4 heads of width 256      (S, H)
    v = (memory @ Wv)
    q = (decoder @ Wq)
    attn = softmax(q k^T / sqrt(D) - gauss, mask -> NEG) * query_mask
    res  = attn @ v                                        (S, H)
    out  = layernorm(concat([decoder, res]) @ Wf + bf + decoder)
    returns (out, attn reshaped (NH*B, S, S))

Sharding: data-parallel over batch B=8 across the 8 NeuronCores (one
batch row per core).  Everything else is computed per-core with no
collectives.

Per-core kernel layout notes:
  - Host passes memory^T and decoder^T (H, S) so QKV projections come out
    in (H, S) "transposed" layout, which feeds the scores matmul and the
    final linear directly.
  - combined = where(mask, NEG, -gauss) is precomputed on host (f32) and
    added to the raw scores on DVE; softmax then needs no max-subtraction
    because logits are bounded (~|10|) and masked entries underflow to 0.
  - exp runs on ACT with fused row-sum (accum_out) giving the softmax
    denominator for free; normalization is a per-partition scalar mul that
    also folds in query_mask.
  - attn (q-major) is transposed 128x128-blockwise on the PE array so the
    attn @ v matmul can contract k on the partition dim.
  - Matmuls use float32r (fp32 bits, relaxed PE precision, 4x faster than
    full fp32).  Flip MM_DT to float32 for full precision.
"""

import os
import numpy as np
from contextlib import ExitStack

import concourse.bass as bass
import concourse.mybir as mybir
import concourse.tile as tile
from concourse import bacc
from concourse.bass_utils import run_bass_kernel_spmd
from concourse.masks import make_identity

B, S, H, NH = 8, 1024, 1024, 4
D = H // NH            # 256 head dim
P = 128                # partitions
ST = S // P            # 8 s-tiles
HT = H // P            # 8 feature-tiles
DT = D // P            # 2 feature-tiles per head
NHALF = 2              # halves of S/H for N=512 matmuls
NEG = np.float32(-(2**32) + 1)

F32 = mybir.dt.float32
MM_DT = mybir.dt.float32r   # matmul operand dtype view

AF = mybir.ActivationFunctionType
ALU = mybir.AluOpType


def _r(ap):
    """Matmul operands are already MM_DT-typed tiles; passthrough."""
    return ap


MAX_PHASE = int(os.environ.get("KMAX_PHASE", "3"))   # debug bisection
SUB = os.environ.get("KSUB", "")                       # phase-1 sub-bisect


def _emit(tc, io):
    nc = tc.nc
    with ExitStack() as g:
        const = g.enter_context(tc.tile_pool(name="const", bufs=1))

        ident = const.tile([P, P], F32)
        make_identity(nc, ident)
        qm_sb = const.tile([P, ST], F32)
        nc.sync.dma_start(qm_sb[:], io["qm"][:])

        # kqv pool spans phases 1-2; res pool spans phases 2-3.  Their
        # lifetimes overlap but neither contains the other, so they are
        # opened/closed manually rather than nested.
        kqv_cm = tc.tile_pool(name="kqv", bufs=1)
        kqv = kqv_cm.__enter__()
        kT = kqv.tile([P, HT, S], MM_DT, tag="kT")
        qT = kqv.tile([P, HT, S], MM_DT, tag="qT")
        vA = kqv.tile([P, ST, H], MM_DT, tag="vA")

        # ---- Phase 1: QKV projections -> kT_all, qT_all (H,S), v_all (S,H)
        with ExitStack() as c1:
            xres = c1.enter_context(tc.tile_pool(name="xres", bufs=1))
            wkq = c1.enter_context(tc.tile_pool(name="wkq", bufs=2))
            wvp = c1.enter_context(tc.tile_pool(name="wvp", bufs=1))
            mmp = c1.enter_context(
                tc.tile_pool(name="mmp", bufs=2, space="PSUM"))

            memT_sb = xres.tile([P, HT, S], MM_DT, tag="memT")
            decT_sb = xres.tile([P, HT, S], MM_DT, tag="decT")
            nc.gpsimd.dma_start(
                memT_sb[:], io["memT"][:].rearrange("(t p) s -> p t s", p=P))
            if SUB == "dma":
                for t in range(HT):
                    nc.sync.dma_start(
                        io["out"][t * P:(t + 1) * P, :],
                        memT_sb[:, t, :].bitcast(F32))

            def kq_proj(w_dram, x_sb, dst):
                for m in range(HT):  # output feature tile
                    wm = wkq.tile([P, HT, P], MM_DT, tag="w")
                    nc.gpsimd.dma_start(
                        wm[:],
                        w_dram[:, m * P:(m + 1) * P]
                        .rearrange("(t p) c -> p t c", p=P))
                    for sh in range(NHALF):
                        if SUB == "dma":
                            continue
                        ps = mmp.tile([P, 512], F32, tag="mm")
                        for t in range(HT):
                            nc.tensor.matmul(
                                ps[:],
                                _r(wm[:, t, :]),
                                _r(x_sb[:, t, sh * 512:(sh + 1) * 512]),
                                start=(t == 0), stop=(t == HT - 1))
                        nc.scalar.copy(
                            dst[:, m, sh * 512:(sh + 1) * 512], ps[:])

            def load_wv(vh):
                wv = wvp.tile([P, HT, 512], MM_DT, tag="wv")
                nc.gpsimd.dma_start(
                    wv[:],
                    io["wv"][:, vh * 512:(vh + 1) * 512]
                    .rearrange("(t p) c -> p t c", p=P))
                return wv

            def v_proj(wv, vh):
                for st in range(ST):
                    if SUB in ("dma", "kq"):
                        continue
                    ps = mmp.tile([P, 512], F32, tag="mm")
                    for t in range(HT):
                        nc.tensor.matmul(
                            ps[:],
                            _r(memT_sb[:, t, st * P:(st + 1) * P]),
                            _r(wv[:, t, :]),
                            start=(t == 0), stop=(t == HT - 1))
                    nc.scalar.copy(
                        vA[:, st, vh * 512:(vh + 1) * 512], ps[:])

            # critical path: memT + first wk tile feed the first matmul.
            # wv0/decT prefetch under the kT matmuls, wv1 under qT.
            kq_proj(io["wk"], memT_sb, kT)
            wv0 = load_wv(0)
            nc.gpsimd.dma_start(
                decT_sb[:], io["decT"][:].rearrange("(t p) s -> p t s", p=P))
            v_proj(wv0, 0)
            wv1 = load_wv(1)
            kq_proj(io["wq"], decT_sb, qT)
            v_proj(wv1, 1)

        if MAX_PHASE < 2:
            if SUB != "dma":
                for t in range(HT):
                    nc.sync.dma_start(
                        io["out"][t * P:(t + 1) * P, :],
                        kT[:, t, :].bitcast(F32))
            kqv_cm.__exit__(None, None, None)
            return

        # ---- Phase 2: attention per (q_super, head)
        res_cm = tc.tile_pool(name="res", bufs=1, side="right")
        resp = res_cm.__enter__()
        resT = resp.tile([P, HT, S], MM_DT, tag="resT")
        with ExitStack() as c2:
            combp = c2.enter_context(tc.tile_pool(name="combp", bufs=5))
            expp = c2.enter_context(tc.tile_pool(name="expp", bufs=3))
            atp = c2.enter_context(tc.tile_pool(name="atp", bufs=1))
            sml = c2.enter_context(tc.tile_pool(name="sml", bufs=6))
            scp = c2.enter_context(
                tc.tile_pool(name="scp", bufs=2, space="PSUM"))
            tpp = c2.enter_context(
                tc.tile_pool(name="tpp", bufs=2, space="PSUM"))
            rsp = c2.enter_context(
                tc.tile_pool(name="rsp", bufs=2, space="PSUM"))

            for qs in range(2):            # 512-row q superblock
                combs = []
                for qb in range(4):
                    qi = qs * 4 + qb
                    cmb = combp.tile([P, S], F32, tag="comb")
                    nc.sync.dma_start(
                        cmb[:], io["comb"][qi * P:(qi + 1) * P, :])
                    combs.append(cmb)
                for h in range(NH):
                    at = atp.tile([P, ST, 512], MM_DT, tag="attnT")
                    for qb in range(4):
                        qi = qs * 4 + qb
                        ps = scp.tile([P, S], F32, tag="sc")
                        for dt_i in range(DT):
                            for kh in range(NHALF):
                                nc.tensor.matmul(
                                    ps[:, kh * 512:(kh + 1) * 512],
                                    _r(qT[:, h * DT + dt_i,
                                          qi * P:(qi + 1) * P]),
                                    _r(kT[:, h * DT + dt_i,
                                          kh * 512:(kh + 1) * 512]),
                                    start=(dt_i == 0), stop=(dt_i == DT - 1))
                        # logits = scores + combined (PSUM+SBUF -> SBUF)
                        lg = expp.tile([P, S], F32, tag="lg")
                        nc.vector.tensor_add(lg[:], ps[:], combs[qb][:])
                        e = expp.tile([P, S], F32, tag="exp")
                        den = sml.tile([P, 1], F32, tag="den")
                        nc.scalar.activation(
                            e[:], lg[:], AF.Exp, accum_out=den[:])
                        rcp = sml.tile([P, 1], F32, tag="rcp")
                        nc.vector.reciprocal(rcp[:], den[:])
                        scl = sml.tile([P, 1], F32, tag="scl")
                        nc.vector.tensor_mul(
                            scl[:], rcp[:], qm_sb[:, qi:qi + 1])
                        nc.vector.tensor_scalar_mul(e[:], e[:], scl[:])
                        nc.sync.dma_start(
                            io["attn_out"][h, qi * P:(qi + 1) * P, :], e[:])
                        # transpose the 8 [128,128] blocks of this q row-block
                        for grp in range(2):
                            tp = tpp.tile([P, 512], F32, tag="tp")
                            for j in range(4):
                                kt = grp * 4 + j
                                nc.tensor.transpose(
                                    tp[:, j * P:(j + 1) * P],
                                    e[:, kt * P:(kt + 1) * P],
                                    ident[:])
                            nc.scalar.copy(
                                at[:, grp * 4:(grp + 1) * 4,
                                   qb * P:(qb + 1) * P],
                                tp[:].rearrange("p (j q) -> p j q", j=4))
                    # attn^T @ ... -> res^T for this (head, q_super)
                    for dm in range(DT):
                        pr = rsp.tile([P, 512], F32, tag="rs")
                        for kt in range(ST):
                            nc.tensor.matmul(
                                pr[:],
                                _r(vA[:, kt,
                                      h * D + dm * P:h * D + (dm + 1) * P]),
                                _r(at[:, kt, :]),
                                start=(kt == 0), stop=(kt == ST - 1))
                        nc.scalar.copy(
                            resT[:, h * DT + dm, qs * 512:(qs + 1) * 512],
                            pr[:])

        kqv_cm.__exit__(None, None, None)

        if MAX_PHASE < 3:
            for t in range(HT):
                nc.sync.dma_start(
                    io["out"][t * P:(t + 1) * P, :],
                    resT[:, t, :].bitcast(F32))
            res_cm.__exit__(None, None, None)
            return

        # ---- Phase 3: final linear + residual + layernorm
        with ExitStack() as c3:
            wfp = c3.enter_context(tc.tile_pool(name="wfp", bufs=1))
            dcp = c3.enter_context(tc.tile_pool(name="dcp", bufs=2))
            drp = c3.enter_context(tc.tile_pool(name="drp", bufs=2))
            osb = c3.enter_context(tc.tile_pool(name="osb", bufs=2))
            sml3 = c3.enter_context(tc.tile_pool(name="sml3", bufs=4))
            opp = c3.enter_context(
                tc.tile_pool(name="opp", bufs=4, space="PSUM"))

            wf_sb = wfp.tile([P, 2 * HT, H], MM_DT, tag="wf")
            for nh2 in range(NHALF):
                for t in range(2 * HT):
                    nc.gpsimd.dma_start(
                        wf_sb[:, t, nh2 * 512:(nh2 + 1) * 512],
                        io["wf"][t * P:(t + 1) * P,
                                 nh2 * 512:(nh2 + 1) * 512])

            for sm in range(ST):
                dcT = dcp.tile([P, HT, P], MM_DT, tag="dcT")
                nc.gpsimd.dma_start(
                    dcT[:],
                    io["decT"][:, sm * P:(sm + 1) * P]
                    .rearrange("(t p) c -> p t c", p=P))
                dr = drp.tile([P, H], F32, tag="dr")
                nc.sync.dma_start(
                    dr[:], io["dec_res"][sm * P:(sm + 1) * P, :])
                ob = osb.tile([P, H], F32, tag="ob")
                sqs = sml3.tile([P, 2], F32, tag="sqs")
                for nh2 in range(NHALF):
                    po = opp.tile([P, 512], F32, tag="po")
                    for t in range(2 * HT):
                        lsrc = dcT[:, t, :] if t < HT else \
                            resT[:, t - HT, sm * P:(sm + 1) * P]
                        nc.tensor.matmul(
                            po[:], _r(lsrc),
                            _r(wf_sb[:, t, nh2 * 512:(nh2 + 1) * 512]),
                            start=(t == 0), stop=(t == 2 * HT - 1))
                    # residual add (plain tensor_tensor; TTR can't read PSUM)
                    nc.vector.tensor_add(
                        ob[:, nh2 * 512:(nh2 + 1) * 512], po[:],
                        dr[:, nh2 * 512:(nh2 + 1) * 512])
                    # sum of squares (Square output is discarded into PSUM)
                    sq_ps = opp.tile([P, 512], F32, tag="po")
                    nc.scalar.activation(
                        sq_ps[:], ob[:, nh2 * 512:(nh2 + 1) * 512],
                        AF.Square, accum_out=sqs[:, nh2:nh2 + 1])
                mu = sml3.tile([P, 1], F32, tag="mu")
                nc.vector.reduce_sum(mu[:], ob[:], axis=mybir.AxisListType.X)
                nc.vector.tensor_scalar_mul(mu[:], mu[:], 1.0 / H)
                var = sml3.tile([P, 1], F32, tag="var")
                nc.vector.tensor_add(var[:], sqs[:, 0:1], sqs[:, 1:2])
                nc.vector.tensor_scalar_mul(var[:], var[:], 1.0 / H)
                mu2 = sml3.tile([P, 1], F32, tag="mu2")
                nc.vector.tensor_mul(mu2[:], mu[:], mu[:])
                nc.vector.tensor_sub(var[:], var[:], mu2[:])
                nc.vector.tensor_scalar_add(var[:], var[:], 1e-5)
                sd = sml3.tile([P, 1], F32, tag="sd")
                nc.scalar.sqrt(sd[:], var[:])
                rstd = sml3.tile([P, 1], F32, tag="rstd")
                nc.vector.reciprocal(rstd[:], sd[:])
                ob2 = osb.tile([P, H], F32, tag="ob2")
                nc.vector.tensor_scalar(
                    ob2[:], ob[:], mu[:], rstd[:],
                    op0=ALU.subtract, op1=ALU.mult)
                nc.sync.dma_start(io["out"][sm * P:(sm + 1) * P, :], ob2[:])

        res_cm.__exit__(None, None, None)


_PROGRAM = None


def _build():
    global _PROGRAM
    if _PROGRAM is not None:
        return _PROGRAM
    nc = bacc.Bacc("TRN2", target_bir_lowering=False, debug=False)
    io = {
        "memT": nc.dram_tensor("memT", [H, S], MM_DT, kind="ExternalInput"),
        "decT": nc.dram_tensor("decT", [H, S], MM_DT, kind="ExternalInput"),
        "dec_res": nc.dram_tensor("dec_res", [S, H], F32,
                                  kind="ExternalInput"),
        "comb": nc.dram_tensor("comb", [S, S], F32, kind="ExternalInput"),
        "qm": nc.dram_tensor("qm", [P, ST], F32, kind="ExternalInput"),
        "wq": nc.dram_tensor("wq", [H, H], MM_DT, kind="ExternalInput"),
        "wk": nc.dram_tensor("wk", [H, H], MM_DT, kind="ExternalInput"),
        "wv": nc.dram_tensor("wv", [H, H], MM_DT, kind="ExternalInput"),
        "wf": nc.dram_tensor("wf", [2 * H, H], MM_DT, kind="ExternalInput"),
        "attn_out": nc.dram_tensor("attn_out", [NH, S, S], F32,
                                   kind="ExternalOutput"),
        "out": nc.dram_tensor("out", [S, H], F32, kind="ExternalOutput"),
    }
    io = {k: (v.ap() if hasattr(v, "ap") else v) for k, v in io.items()}
    with tile.TileContext(nc) as tc:
        _emit(tc, io)
    nc.compile()
    _PROGRAM = nc
    return nc


def _host_prep(memory, decoder_input, mask, query_mask, Wk, Wv, Wq, Wf, bf,
               g_factor):
    """Build the per-core input maps."""
    memory = np.ascontiguousarray(np.asarray(memory, np.float32))
    decoder_input = np.ascontiguousarray(np.asarray(decoder_input, np.float32))
    mask = np.asarray(mask, bool)
    query_mask = np.asarray(query_mask, np.float32)
    Wk = np.ascontiguousarray(np.asarray(Wk, np.float32))
    Wv = np.ascontiguousarray(np.asarray(Wv, np.float32))
    Wq = np.ascontiguousarray(np.asarray(Wq, np.float32))
    Wf = np.ascontiguousarray(np.asarray(Wf, np.float32))
    bf = np.asarray(bf, np.float32)
    g = float(np.asarray(g_factor, np.float32).reshape(-1)[0])

    idx = np.arange(S, dtype=np.float32)
    gauss = (idx[:, None] - idx[None, :]) ** 2 / np.float32(g)
    wq_s = (Wq / np.float32(np.sqrt(D))).astype(np.float32)

    in_maps = []
    for b in range(B):
        comb = np.where(mask[b], NEG, -gauss).astype(np.float32)
        in_maps.append({
            "memT": np.ascontiguousarray(memory[b].T),
            "decT": np.ascontiguousarray(decoder_input[b].T),
            "dec_res": (decoder_input[b] + bf[None, :]).astype(np.float32),
            "comb": comb,
            "qm": np.ascontiguousarray(
                query_mask[b].reshape(ST, P).T.astype(np.float32)),
            "wq": wq_s,
            "wk": Wk,
            "wv": Wv,
            "wf": Wf,
        })
    return in_maps


def kernel(memory, decoder_input, mask, query_mask, Wk, Wv, Wq, Wf, bf,
           gamma, beta, g_factor, _trace=False, _trace_kwargs=None):
    nc = _build()
    in_maps = _host_prep(memory, decoder_input, mask, query_mask,
                         Wk, Wv, Wq, Wf, bf, g_factor)
    res = run_bass_kernel_spmd(
        nc, in_maps, list(range(B)), trace=_trace,
        **(_trace_kwargs or {}))

    gamma = np.asarray(gamma, np.float32)
    beta = np.asarray(beta, np.float32)
    out = np.empty((B, S, H), np.float32)
    attn = np.empty((NH * B, S, S), np.float32)
    for b in range(B):
        o = res.results[b]["out"]
        if not (np.all(gamma == 1.0) and np.all(beta == 0.0)):
            o = o * gamma[None, :] + beta[None, :]
        out[b] = o
        for h in range(NH):
            attn[h * B + b] = res.results[b]["attn_out"][h]
    kernel._last_results = res
    return out, attn


# revision 18
# speedup vs baseline: 1.0516x; 1.0516x over previous
"""Trainium2 Bass kernel for nn_Attention_85882166051391.

Reference computation (per batch b):
    k = (memory @ Wk)  viewed as 4 heads of width 256      (S, H)
    v = (memory @ Wv)
    q = (decoder @ Wq)
    attn = softmax(q k^T / sqrt(D) - gauss, mask -> NEG) * query_mask
    res  = attn @ v                                        (S, H)
    out  = layernorm(concat([decoder, res]) @ Wf + bf + decoder)
    returns (out, attn reshaped (NH*B, S, S))

Sharding: data-parallel over batch B=8 across the 8 NeuronCores (one
batch row per core).  Everything else is computed per-core with no
collectives.

Per-core kernel layout notes:
  - Host passes memory^T and decoder^T (H, S) so QKV projections come out
    in (H, S) "transposed" layout, which feeds the scores matmul and the
    final linear directly.
  - combined = where(mask, NEG, -gauss) is precomputed on host (f32) and
    added to the raw scores on DVE; softmax then needs no max-subtraction
    because logits are bounded (~|10|) and masked entries underflow to 0.
  - exp runs on ACT with fused row-sum (accum_out) giving the softmax
    denominator for free; normalization is a per-partition scalar mul that
    also folds in query_mask.
  - attn (q-major) is transposed 128x128-blockwise on the PE array so the
    attn @ v matmul can contract k on the partition dim.
  - Matmuls use float32r (fp32 bits, relaxed PE precision, 4x faster than
    full fp32).  Flip MM_DT to float32 for full precision.
"""

import os
import numpy as np
from contextlib import ExitStack

import concourse.bass as bass
import concourse.mybir as mybir
import concourse.tile as tile
from concourse import bacc
from concourse.bass_utils import run_bass_kernel_spmd
from concourse.masks import make_identity

B, S, H, NH = 8, 1024, 1024, 4
D = H // NH            # 256 head dim
P = 128                # partitions
ST = S // P            # 8 s-tiles
HT = H // P            # 8 feature-tiles
DT = D // P            # 2 feature-tiles per head
NHALF = 2              # halves of S/H for N=512 matmuls
NEG = np.float32(-(2**32) + 1)

F32 = mybir.dt.float32
MM_DT = mybir.dt.float32r   # matmul operand dtype view

AF = mybir.ActivationFunctionType
ALU = mybir.AluOpType


def _r(ap):
    """Matmul operands are already MM_DT-typed tiles; passthrough."""
    return ap


MAX_PHASE = int(os.environ.get("KMAX_PHASE", "3"))   # debug bisection
SUB = os.environ.get("KSUB", "")                       # phase-1 sub-bisect


def _emit(tc, io):
    nc = tc.nc
    with ExitStack() as g:
        const = g.enter_context(tc.tile_pool(name="const", bufs=1))

        ident = const.tile([P, P], F32)
        make_identity(nc, ident)
        qm_sb = const.tile([P, ST], F32)
        nc.sync.dma_start(qm_sb[:], io["qm"][:])

        # kqv pool spans phases 1-2; res pool spans phases 2-3.  Their
        # lifetimes overlap but neither contains the other, so they are
        # opened/closed manually rather than nested.
        kqv_cm = tc.tile_pool(name="kqv", bufs=1)
        kqv = kqv_cm.__enter__()
        kT = kqv.tile([P, HT, S], MM_DT, tag="kT")
        qT = kqv.tile([P, HT, S], MM_DT, tag="qT")
        vA = kqv.tile([P, ST, H], MM_DT, tag="vA")

        # ---- Phase 1: QKV projections -> kT_all, qT_all (H,S), v_all (S,H)
        with ExitStack() as c1:
            xres = c1.enter_context(tc.tile_pool(name="xres", bufs=1))
            wkq = c1.enter_context(tc.tile_pool(name="wkq", bufs=2))
            wvp = c1.enter_context(tc.tile_pool(name="wvp", bufs=1))
            mmp = c1.enter_context(
                tc.tile_pool(name="mmp", bufs=2, space="PSUM"))

            memT_sb = xres.tile([P, HT, S], MM_DT, tag="memT")
            decT_sb = xres.tile([P, HT, S], MM_DT, tag="decT")
            for t in range(HT):
                nc.gpsimd.dma_start(
                    memT_sb[:, t, :], io["memT"][t * P:(t + 1) * P, :])
            if SUB == "dma":
                for t in range(HT):
                    nc.sync.dma_start(
                        io["out"][t * P:(t + 1) * P, :],
                        memT_sb[:, t, :].bitcast(F32))

            def kq_proj(w_dram, x_sb, dst):
                for m in range(HT):  # output feature tile
                    wm = wkq.tile([P, HT, P], MM_DT, tag="w")
                    nc.gpsimd.dma_start(
                        wm[:],
                        w_dram[:, m * P:(m + 1) * P]
                        .rearrange("(t p) c -> p t c", p=P))
                    for sh in range(NHALF):
                        if SUB == "dma":
                            continue
                        ps = mmp.tile([P, 512], F32, tag="mm")
                        for t in range(HT):
                            nc.tensor.matmul(
                                ps[:],
                                _r(wm[:, t, :]),
                                _r(x_sb[:, t, sh * 512:(sh + 1) * 512]),
                                start=(t == 0), stop=(t == HT - 1))
                        nc.scalar.copy(
                            dst[:, m, sh * 512:(sh + 1) * 512], ps[:])

            def load_wv(vh):
                wv = wvp.tile([P, HT, 512], MM_DT, tag="wv")
                nc.gpsimd.dma_start(
                    wv[:],
                    io["wv"][:, vh * 512:(vh + 1) * 512]
                    .rearrange("(t p) c -> p t c", p=P))
                return wv

            def v_proj(wv, vh):
                for st in range(ST):
                    if SUB in ("dma", "kq"):
                        continue
                    ps = mmp.tile([P, 512], F32, tag="mm")
                    for t in range(HT):
                        nc.tensor.matmul(
                            ps[:],
                            _r(memT_sb[:, t, st * P:(st + 1) * P]),
                            _r(wv[:, t, :]),
                            start=(t == 0), stop=(t == HT - 1))
                    nc.scalar.copy(
                        vA[:, st, vh * 512:(vh + 1) * 512], ps[:])

            # critical path: memT + first wk tile feed the first matmul.
            # wv0/decT prefetch under the kT matmuls, wv1 under qT.
            kq_proj(io["wk"], memT_sb, kT)
            wv0 = load_wv(0)
            for t in range(HT):
                nc.sync.dma_start(
                    decT_sb[:, t, :], io["decT"][t * P:(t + 1) * P, :])
            v_proj(wv0, 0)
            wv1 = load_wv(1)
            kq_proj(io["wq"], decT_sb, qT)
            v_proj(wv1, 1)

        if MAX_PHASE < 2:
            if SUB != "dma":
                for t in range(HT):
                    nc.sync.dma_start(
                        io["out"][t * P:(t + 1) * P, :],
                        kT[:, t, :].bitcast(F32))
            kqv_cm.__exit__(None, None, None)
            return

        # ---- Phase 2: attention per (q_super, head)
        res_cm = tc.tile_pool(name="res", bufs=1, side="right")
        resp = res_cm.__enter__()
        resT = resp.tile([P, HT, S], MM_DT, tag="resT")
        with ExitStack() as c2:
            combp = c2.enter_context(tc.tile_pool(name="combp", bufs=5))
            expp = c2.enter_context(tc.tile_pool(name="expp", bufs=3))
            atp = c2.enter_context(tc.tile_pool(name="atp", bufs=1))
            sml = c2.enter_context(tc.tile_pool(name="sml", bufs=6))
            scp = c2.enter_context(
                tc.tile_pool(name="scp", bufs=2, space="PSUM"))
            tpp = c2.enter_context(
                tc.tile_pool(name="tpp", bufs=2, space="PSUM"))
            rsp = c2.enter_context(
                tc.tile_pool(name="rsp", bufs=2, space="PSUM"))

            for qs in range(2):            # 512-row q superblock
                combs = []
                for qb in range(4):
                    qi = qs * 4 + qb
                    cmb = combp.tile([P, S], F32, tag="comb")
                    nc.sync.dma_start(
                        cmb[:], io["comb"][qi * P:(qi + 1) * P, :])
                    combs.append(cmb)
                for h in range(NH):
                    at = atp.tile([P, ST, 512], MM_DT, tag="attnT")
                    for qb in range(4):
                        qi = qs * 4 + qb
                        ps = scp.tile([P, S], F32, tag="sc")
                        for dt_i in range(DT):
                            for kh in range(NHALF):
                                nc.tensor.matmul(
                                    ps[:, kh * 512:(kh + 1) * 512],
                                    _r(qT[:, h * DT + dt_i,
                                          qi * P:(qi + 1) * P]),
                                    _r(kT[:, h * DT + dt_i,
                                          kh * 512:(kh + 1) * 512]),
                                    start=(dt_i == 0), stop=(dt_i == DT - 1))
                        # logits = scores + combined (PSUM+SBUF -> SBUF)
                        lg = expp.tile([P, S], F32, tag="lg")
                        nc.vector.tensor_add(lg[:], ps[:], combs[qb][:])
                        e = expp.tile([P, S], F32, tag="exp")
                        den = sml.tile([P, 1], F32, tag="den")
                        nc.scalar.activation(
                            e[:], lg[:], AF.Exp, accum_out=den[:])
                        rcp = sml.tile([P, 1], F32, tag="rcp")
                        nc.vector.reciprocal(rcp[:], den[:])
                        scl = sml.tile([P, 1], F32, tag="scl")
                        nc.vector.tensor_mul(
                            scl[:], rcp[:], qm_sb[:, qi:qi + 1])
                        nc.vector.tensor_scalar_mul(e[:], e[:], scl[:])
                        nc.sync.dma_start(
                            io["attn_out"][h, qi * P:(qi + 1) * P, :], e[:])
                        # transpose the 8 [128,128] blocks of this q row-block
                        for grp in range(2):
                            tp = tpp.tile([P, 512], F32, tag="tp")
                            for j in range(4):
                                kt = grp * 4 + j
                                nc.tensor.transpose(
                                    tp[:, j * P:(j + 1) * P],
                                    e[:, kt * P:(kt + 1) * P],
                                    ident[:])
                            nc.scalar.copy(
                                at[:, grp * 4:(grp + 1) * 4,
                                   qb * P:(qb + 1) * P],
                                tp[:].rearrange("p (j q) -> p j q", j=4))
                    # attn^T @ ... -> res^T for this (head, q_super)
                    for dm in range(DT):
                        pr = rsp.tile([P, 512], F32, tag="rs")
                        for kt in range(ST):
                            nc.tensor.matmul(
                                pr[:],
                                _r(vA[:, kt,
                                      h * D + dm * P:h * D + (dm + 1) * P]),
                                _r(at[:, kt, :]),
                                start=(kt == 0), stop=(kt == ST - 1))
                        nc.scalar.copy(
                            resT[:, h * DT + dm, qs * 512:(qs + 1) * 512],
                            pr[:])

        kqv_cm.__exit__(None, None, None)

        if MAX_PHASE < 3:
            for t in range(HT):
                nc.sync.dma_start(
                    io["out"][t * P:(t + 1) * P, :],
                    resT[:, t, :].bitcast(F32))
            res_cm.__exit__(None, None, None)
            return

        # ---- Phase 3: final linear + residual + layernorm
        with ExitStack() as c3:
            wfp = c3.enter_context(tc.tile_pool(name="wfp", bufs=1))
            dcp = c3.enter_context(tc.tile_pool(name="dcp", bufs=2))
            drp = c3.enter_context(tc.tile_pool(name="drp", bufs=2))
            osb = c3.enter_context(tc.tile_pool(name="osb", bufs=2))
            sml3 = c3.enter_context(tc.tile_pool(name="sml3", bufs=4))
            opp = c3.enter_context(
                tc.tile_pool(name="opp", bufs=4, space="PSUM"))

            wf_sb = wfp.tile([P, 2 * HT, H], MM_DT, tag="wf")
            for nh2 in range(NHALF):
                for t in range(2 * HT):
                    eng = nc.gpsimd if (t % 2 == 0) else nc.sync
                    eng.dma_start(
                        wf_sb[:, t, nh2 * 512:(nh2 + 1) * 512],
                        io["wf"][t * P:(t + 1) * P,
                                 nh2 * 512:(nh2 + 1) * 512])

            for sm in range(ST):
                dcT = dcp.tile([P, HT, P], MM_DT, tag="dcT")
                nc.scalar.dma_start(
                    dcT[:],
                    io["decT"][:, sm * P:(sm + 1) * P]
                    .rearrange("(t p) c -> p t c", p=P))
                dr = drp.tile([P, H], F32, tag="dr")
                nc.sync.dma_start(
                    dr[:], io["dec_res"][sm * P:(sm + 1) * P, :])
                ob = osb.tile([P, H], F32, tag="ob")
                sqs = sml3.tile([P, 2], F32, tag="sqs")
                for nh2 in range(NHALF):
                    po = opp.tile([P, 512], F32, tag="po")
                    for t in range(2 * HT):
                        lsrc = dcT[:, t, :] if t < HT else \
                            resT[:, t - HT, sm * P:(sm + 1) * P]
                        nc.tensor.matmul(
                            po[:], _r(lsrc),
                            _r(wf_sb[:, t, nh2 * 512:(nh2 + 1) * 512]),
                            start=(t == 0), stop=(t == 2 * HT - 1))
                    # residual add (plain tensor_tensor; TTR can't read PSUM)
                    nc.vector.tensor_add(
                        ob[:, nh2 * 512:(nh2 + 1) * 512], po[:],
                        dr[:, nh2 * 512:(nh2 + 1) * 512])
                    # sum of squares (Square output is discarded into PSUM)
                    sq_ps = opp.tile([P, 512], F32, tag="po")
                    nc.scalar.activation(
                        sq_ps[:], ob[:, nh2 * 512:(nh2 + 1) * 512],
                        AF.Square, accum_out=sqs[:, nh2:nh2 + 1])
                mu = sml3.tile([P, 1], F32, tag="mu")
                nc.vector.reduce_sum(mu[:], ob[:], axis=mybir.AxisListType.X)
                nc.vector.tensor_scalar_mul(mu[:], mu[:], 1.0 / H)
                var = sml3.tile([P, 1], F32, tag="var")
                nc.vector.tensor_add(var[:], sqs[:, 0:1], sqs[:, 1:2])
                nc.vector.tensor_scalar_mul(var[:], var[:], 1.0 / H)
                mu2 = sml3.tile([P, 1], F32, tag="mu2")
                nc.vector.tensor_mul(mu2[:], mu[:], mu[:])
                nc.vector.tensor_sub(var[:], var[:], mu2[:])
                nc.vector.tensor_scalar_add(var[:], var[:], 1e-5)
                sd = sml3.tile([P, 1], F32, tag="sd")
                nc.scalar.sqrt(sd[:], var[:])
                rstd = sml3.tile([P, 1], F32, tag="rstd")
                nc.vector.reciprocal(rstd[:], sd[:])
                ob2 = osb.tile([P, H], F32, tag="ob2")
                nc.vector.tensor_scalar(
                    ob2[:], ob[:], mu[:], rstd[:],
                    op0=ALU.subtract, op1=ALU.mult)
                nc.sync.dma_start(io["out"][sm * P:(sm + 1) * P, :], ob2[:])

        res_cm.__exit__(None, None, None)


_PROGRAM = None


def _build():
    global _PROGRAM
    if _PROGRAM is not None:
        return _PROGRAM
    nc = bacc.Bacc("TRN2", target_bir_lowering=False, debug=False)
    io = {
        "memT": nc.dram_tensor("memT", [H, S], MM_DT, kind="ExternalInput"),
        "decT": nc.dram_tensor("decT", [H, S], MM_DT, kind="ExternalInput"),
        "dec_res": nc.dram_tensor("dec_res", [S, H], F32,
                                  kind="ExternalInput"),
        "comb": nc.dram_tensor("comb", [S, S], F32, kind="ExternalInput"),
        "qm": nc.dram_tensor("qm", [P, ST], F32, kind="ExternalInput"),
        "wq": nc.dram_tensor("wq", [H, H], MM_DT, kind="ExternalInput"),
        "wk": nc.dram_tensor("wk", [H, H], MM_DT, kind="ExternalInput"),
        "wv": nc.dram_tensor("wv", [H, H], MM_DT, kind="ExternalInput"),
        "wf": nc.dram_tensor("wf", [2 * H, H], MM_DT, kind="ExternalInput"),
        "attn_out": nc.dram_tensor("attn_out", [NH, S, S], F32,
                                   kind="ExternalOutput"),
        "out": nc.dram_tensor("out", [S, H], F32, kind="ExternalOutput"),
    }
    io = {k: (v.ap() if hasattr(v, "ap") else v) for k, v in io.items()}
    with tile.TileContext(nc) as tc:
        _emit(tc, io)
    nc.compile()
    _PROGRAM = nc
    return nc


def _host_prep(memory, decoder_input, mask, query_mask, Wk, Wv, Wq, Wf, bf,
               g_factor):
    """Build the per-core input maps."""
    memory = np.ascontiguousarray(np.asarray(memory, np.float32))
    decoder_input = np.ascontiguousarray(np.asarray(decoder_input, np.float32))
    mask = np.asarray(mask, bool)
    query_mask = np.asarray(query_mask, np.float32)
    Wk = np.ascontiguousarray(np.asarray(Wk, np.float32))
    Wv = np.ascontiguousarray(np.asarray(Wv, np.float32))
    Wq = np.ascontiguousarray(np.asarray(Wq, np.float32))
    Wf = np.ascontiguousarray(np.asarray(Wf, np.float32))
    bf = np.asarray(bf, np.float32)
    g = float(np.asarray(g_factor, np.float32).reshape(-1)[0])

    idx = np.arange(S, dtype=np.float32)
    gauss = (idx[:, None] - idx[None, :]) ** 2 / np.float32(g)
    wq_s = (Wq / np.float32(np.sqrt(D))).astype(np.float32)

    in_maps = []
    for b in range(B):
        comb = np.where(mask[b], NEG, -gauss).astype(np.float32)
        in_maps.append({
            "memT": np.ascontiguousarray(memory[b].T),
            "decT": np.ascontiguousarray(decoder_input[b].T),
            "dec_res": (decoder_input[b] + bf[None, :]).astype(np.float32),
            "comb": comb,
            "qm": np.ascontiguousarray(
                query_mask[b].reshape(ST, P).T.astype(np.float32)),
            "wq": wq_s,
            "wk": Wk,
            "wv": Wv,
            "wf": Wf,
        })
    return in_maps


def kernel(memory, decoder_input, mask, query_mask, Wk, Wv, Wq, Wf, bf,
           gamma, beta, g_factor, _trace=False, _trace_kwargs=None):
    nc = _build()
    in_maps = _host_prep(memory, decoder_input, mask, query_mask,
                         Wk, Wv, Wq, Wf, bf, g_factor)
    res = run_bass_kernel_spmd(
        nc, in_maps, list(range(B)), trace=_trace,
        **(_trace_kwargs or {}))

    gamma = np.asarray(gamma, np.float32)
    beta = np.asarray(beta, np.float32)
    out = np.empty((B, S, H), np.float32)
    attn = np.empty((NH * B, S, S), np.float32)
    for b in range(B):
        o = res.results[b]["out"]
        if not (np.all(gamma == 1.0) and np.all(beta == 0.0)):
            o = o * gamma[None, :] + beta[None, :]
        out[b] = o
        for h in range(NH):
            attn[h * B + b] = res.results[b]["attn_out"][h]
    kernel._last_results = res
    return out, attn
